# revision 29
# baseline (speedup 1.0000x reference)
"""Multi-head self-attention block on 8 trn2 NeuronCores.

Strategy: tensor-parallel over heads (16 heads -> 2 per core). Each core
computes QKV + attention for its 2 heads over ALL tokens, then a PARTIAL
output projection (contracting only its 128 head-dims of Wout) for all
tokens. The 8 partial outputs are summed on the host — no collective.

v5: V produced directly transposed (no PE transposes); QKV/proj work is
split into small quanta interleaved into the attention kt loop so the PE
queue always has ready work behind exp-waiting instructions; AV pipeline
deepened to 2 kts; norm muls on GpSimd.
"""
import sys
sys.path.insert(0, "/opt/trn_rl_repo")

import numpy as np
import ml_dtypes

import concourse.bass as bass
import concourse.mybir as mybir
import concourse.tile as tile
from concourse import bacc
from concourse import library_config
from concourse.bass_utils import run_bass_kernel_spmd

# Problem shape (hardcoded per contract)
N, T, D, H = 4, 2048, 1024, 16
DK = D // H          # 64
NC = 8               # cores
HPC = H // NC        # 2 heads per core
NT = N * T           # 8192 tokens
TCH = 512            # token chunk for QKV projection matmuls
KT_PER_N = T // 128  # 16 key tiles per batch
QC_PER_N = T // 512  # 4 query chunks of 512 per batch

F32 = mybir.dt.float32
BF16 = mybir.dt.bfloat16

FT = mybir.ActivationFunctionType


def build_bass():
    nc = bacc.Bacc("TRN2", target_bir_lowering=False, debug=False, num_devices=NC)

    zT = nc.dram_tensor("zT", [D, NT], BF16, kind="ExternalInput")
    wq = nc.dram_tensor("wq", [D, HPC * DK], BF16, kind="ExternalInput")
    wk = nc.dram_tensor("wk", [D, HPC * DK], BF16, kind="ExternalInput")
    wv = nc.dram_tensor("wv", [D, HPC * DK], BF16, kind="ExternalInput")
    woutc = nc.dram_tensor("woutc", [HPC * DK, D], BF16, kind="ExternalInput")
    tri = nc.dram_tensor("tri", [128, 128], BF16, kind="ExternalInput")
    outT = nc.dram_tensor("outT", [8, 128, NT], BF16, kind="ExternalOutput")

    zT_v = zT.rearrange("(c p) t -> p c t", p=128)     # [128, 8, NT]
    wq_v = wq.rearrange("(c p) m -> p c m", p=128)     # [128, 8, 128]
    wk_v = wk.rearrange("(c p) m -> p c m", p=128)
    wv_v = wv.rearrange("(c p) m -> p c m", p=128)
    woutc_v = woutc.rearrange("p (o m) -> p o m", o=8)  # [128, 8, 128]
    outT_v = outT.rearrange("o p t -> p o t")           # [128, 8, NT]

    with tile.TileContext(nc) as tc:
        _build_body(nc, tc, zT_v, wq_v, wk_v, wv_v, woutc_v, tri, outT_v)
    nc.compile()
    return nc


def _build_body(nc, tc, zT_v, wq_v, wk_v, wv_v, woutc_v, tri, outT_v):
    import contextlib
    ctx = contextlib.ExitStack()
    with ctx:
        consts = ctx.enter_context(tc.tile_pool(name="consts", bufs=1))
        zpool = ctx.enter_context(tc.tile_pool(name="zpool", bufs=3))
        qkpool = ctx.enter_context(tc.tile_pool(name="qkpool", bufs=2))
        vpool = ctx.enter_context(tc.tile_pool(name="vpool", bufs=2))
        expool = ctx.enter_context(tc.tile_pool(name="expool", bufs=2))
        smalls = ctx.enter_context(tc.tile_pool(name="smalls", bufs=4))
        outpool = ctx.enter_context(tc.tile_pool(name="outpool", bufs=2))
        # PSUM budget: scores 2x2 + av 2 + mm 2 = 8 banks
        ps_sc = ctx.enter_context(tc.tile_pool(name="ps_sc", bufs=1, space="PSUM"))
        ps_av = ctx.enter_context(tc.tile_pool(name="ps_av", bufs=1, space="PSUM"))
        ps_mm = ctx.enter_context(tc.tile_pool(name="ps_mm", bufs=2, space="PSUM"))
        dram = ctx.enter_context(tc.tile_pool(name="dram", bufs=1, space="DRAM"))

        # gpsimd library for tensor_tensor (norm muls)
        nc.gpsimd.load_library(library_config.proxy)

        # ---- constants (spread across queues so nothing serializes) ----
        wq_sb = consts.tile([128, 8, 128], BF16, tag="wq")
        wk_sb = consts.tile([128, 8, 128], BF16, tag="wk")
        wv_sb = consts.tile([128, 8, 128], BF16, tag="wv")
        nc.sync.dma_start(out=wq_sb, in_=wq_v)
        tri_sb = consts.tile([128, 128], BF16, tag="tri")
        nc.gpsimd.dma_start(out=tri_sb, in_=tri[:, :])
        wout_sb = consts.tile([128, 8, 128], BF16, tag="wout")
        nc.gpsimd.dma_start(out=wout_sb, in_=woutc_v)
        ones_row = consts.tile([1, 64], BF16, tag="ones_row")
        nc.vector.memset(ones_row, 1.0)
        # attention output (transposed): rows = 2 local heads x 64, cols = tokens
        attnT = consts.tile([128, NT], BF16, tag="attnT")

        qkv_state = {}

        # ---- filler work queue: small PE quanta interleaved into the
        # attention kt loop so the in-order PE queue always has ready work
        # behind exp-waiting instructions ----
        work = []

        def drain(k):
            for _ in range(k):
                if work:
                    work.pop(0)()

        def drain_all():
            while work:
                work.pop(0)()

        def _qkv_start(n):
            qt = qkpool.tile([128, T], BF16, tag="qt", name=f"qt{n}")
            kt_sb = qkpool.tile([128, T], BF16, tag="kt", name=f"kt{n}")
            vsb = vpool.tile([128, KT_PER_N, HPC, 65], BF16, tag="v",
                             name=f"v{n}")
            nc.vector.memset(vsb[:, :, :, 64:65], 1.0)
            qkv_state[n] = (qt, kt_sb, vsb)

        def _enqueue_chunk(n, tci, split=False, queue=None):
            """Issue z DMA now; enqueue q/k/v projection quanta."""
            tok0 = n * T
            qt, kt_sb, vsb = qkv_state[n]
            zch = zpool.tile([128, 8, TCH], BF16, tag="z", name="zch")
            src = zT_v[:, :, tok0 + tci * TCH: tok0 + (tci + 1) * TCH]
            if split:
                for dc in range(8):
                    eng = nc.sync if dc % 2 == 0 else nc.scalar
                    eng.dma_start(out=zch[:, dc, :], in_=src[:, dc, :])
            else:
                (queue or nc.sync).dma_start(out=zch, in_=src)

            def _proj_qk(w_sb, dst):
                def go():
                    ps = ps_mm.tile([128, 512], F32, tag="mm", name="psqk")
                    for dc in range(8):
                        nc.tensor.matmul(
                            ps[:, :TCH], lhsT=w_sb[:, dc, :],
                            rhs=zch[:, dc, :],
                            start=(dc == 0), stop=(dc == 7))
                    nc.vector.tensor_copy(dst[:, tci * TCH:(tci + 1) * TCH],
                                          ps[:, :TCH])
                return go

            def _proj_v():
                # v directly transposed: out [tokens, (h dk)]; 4 token tiles
                # share one PSUM bank
                psv = ps_mm.tile([128, 4, 128], F32, tag="mm", name="psv")
                for sub in range(4):
                    zsl = slice(sub * 128, (sub + 1) * 128)
                    for dc in range(8):
                        nc.tensor.matmul(
                            psv[:, sub, :], lhsT=zch[:, dc, zsl],
                            rhs=wv_sb[:, dc, :],
                            start=(dc == 0), stop=(dc == 7))
                for sub in range(4):
                    kt_idx = tci * 4 + sub
                    nc.vector.tensor_copy(
                        vsb[:, kt_idx, :, 0:64],
                        psv[:, sub, :].rearrange("p (h m) -> p h m", h=HPC))

            work.append(_proj_qk(wq_sb, qt))
            work.append(_proj_qk(wk_sb, kt_sb))
            work.append(_proj_v)

        # proj quanta are rationed via a backlog so the later (larger,
        # exp-bound) attention steps still have PE filler available
        proj_backlog = []

        def _enqueue_proj(n, qc):
            """Enqueue output-projection quanta for an already-normed qc."""
            tok0 = n * T + qc * 512
            ob = outpool.tile([128, 8, 512], BF16, tag="ob", name="ob",
                              bufs=4)

            def _proj_pair(ot):
                def go():
                    for o in (ot, ot + 1):
                        ps = ps_mm.tile([128, 512], F32, tag="mm",
                                        name="psproj")
                        nc.tensor.matmul(
                            ps, lhsT=wout_sb[:, o, :],
                            rhs=attnT[:, tok0:tok0 + 512],
                            start=True, stop=True)
                        nc.vector.tensor_copy(ob[:, o, :], ps)
                    nc.gpsimd.dma_start(
                        out=outT_v[:, ot:ot + 2, tok0:tok0 + 512],
                        in_=ob[:, ot:ot + 2, :])
                return go

            for ot in range(0, 8, 2):
                proj_backlog.append(_proj_pair(ot))

        def _attn_qc(n, qc):
            qt, kt_sb, vsb = qkv_state[n]
            q0 = qc * 512
            n_kt = 4 * qc + 4
            av = [ps_av.tile([65, 512], F32, tag=f"av{h}", name=f"av{h}")
                  for h in range(HPC)]

            def _issue_av(kt, ex, s):
                for h in range(HPC):
                    nc.tensor.matmul(
                        av[h][:, s:], lhsT=vsb[:, kt, h, :], rhs=ex[:, h, s:],
                        start=(kt == 0), stop=(kt == n_kt - 1))

            # filler quanta: spread evenly over the whole kt loop so late
            # kts are not starved (PE idle there trips the HAM re-throttle)
            n_work = len(work)
            drained = 0

            # 2-deep software pipeline: AV(kt) issues after scores(kt+2) so
            # exp(kt) on ScalarE has ~2 kts of slack
            pend = []
            for kt in range(n_kt):
                # columns [0, s) of this kt row-block are fully causal-masked
                d = kt - 4 * qc
                s = 128 * d if d > 0 else 0
                sc = ps_sc.tile([128, 2, 512], F32, tag="sc", name="sc", bufs=2)
                for h in range(HPC):
                    nc.tensor.matmul(
                        sc[:, h, s:],
                        lhsT=kt_sb[h * 64:(h + 1) * 64,
                                   kt * 128:(kt + 1) * 128],
                        rhs=qt[h * 64:(h + 1) * 64, q0 + s:q0 + 512],
                        start=True, stop=True)
                if len(pend) >= 2:
                    _issue_av(*pend.pop(0))
                ex = expool.tile([128, 2, 512], BF16, tag="ex", name="ex",
                                 bufs=4)
                nc.scalar.activation(ex[:, :, s:], sc[:, :, s:], FT.Exp)
                if d >= 0:
                    # only the 128-col diagonal slab needs the triangle mask
                    for h in range(HPC):
                        nc.vector.tensor_mul(
                            ex[:, h, s:s + 128], ex[:, h, s:s + 128], tri_sb)
                pend.append((kt, ex, s))
                target = ((kt + 1) * n_work + n_kt - 1) // n_kt
                drain(target - drained)
                drained = target
            for p in pend:
                _issue_av(*p)
            avfs = []
            for h in range(HPC):
                avf = smalls.tile([65, 512], F32, tag=f"avf{h}",
                                  name=f"avf{h}", bufs=2)
                nc.vector.tensor_copy(avf, av[h])
                avfs.append(avf)
            return av, avfs

        def _attn_norm(n, qc, av, avfs):
            tok0 = n * T
            q0 = qc * 512
            # batch-end: little attention filler follows, so avoid the DMA
            # round-trip latency via the on-chip broadcast path
            last = (qc == QC_PER_N - 1)
            for h in range(HPC):
                avf = avfs[h]
                if last:
                    den0 = smalls.tile([1, 512], F32, tag=f"den0{h}", bufs=2,
                                       name=f"den0{h}")
                    nc.vector.tensor_copy(den0, avf[64:65, :])
                    rd = smalls.tile([1, 512], F32, tag=f"rdf{h}", bufs=2,
                                     name=f"rdf{h}")
                    nc.vector.reciprocal_approx_fast(rd, den0)
                    rd_bf = smalls.tile([1, 512], BF16, tag=f"rdbf{h}",
                                        bufs=2, name=f"rdbf{h}")
                    nc.vector.tensor_copy(rd_bf, rd)
                    nc.tensor.matmul(av[h][0:64, :], lhsT=ones_row,
                                     rhs=rd_bf, start=True, stop=True)
                    nc.vector.tensor_mul(
                        attnT[h * 64:(h + 1) * 64,
                              tok0 + q0: tok0 + q0 + 512],
                        avf[0:64, :], av[h][0:64, :])
                    continue
                dscr = dram.tile([1, 512], F32, tag=f"dscr{h}", bufs=2,
                                 name=f"dscr{h}")
                nc.sync.dma_start(out=dscr, in_=avf[64:65, :])
                denb = smalls.tile([64, 512], F32, tag=f"denb{h}", bufs=2,
                                   name=f"denb{h}")
                nc.sync.dma_start(out=denb, in_=dscr.to_broadcast([64, 512]))
                rdb = smalls.tile([64, 512], F32, tag=f"rdb{h}", bufs=2,
                                  name=f"rdb{h}")
                nc.vector.reciprocal_approx_fast(rdb, denb)
                nc.gpsimd.tensor_mul(
                    attnT[h * 64:(h + 1) * 64, tok0 + q0: tok0 + q0 + 512],
                    avf[0:64, :], rdb)

        def _proj_last(n, qc):
            # final chunk: emit directly, use freed scores PSUM banks to
            # deepen the evacuation pipeline
            tok0 = n * T + qc * 512
            ob = outpool.tile([128, 8, 512], BF16, tag="ob", name="ob",
                              bufs=4)
            slots = []
            for j in range(2):
                t = ps_sc.tile([128, 2, 512], F32, tag="sc",
                               name=f"pp{j}", bufs=2)
                slots += [t[:, 0, :], t[:, 1, :]]
            for ot in range(8):
                if ot < len(slots):
                    ps = slots[ot]
                else:
                    ps = ps_mm.tile([128, 512], F32, tag="mm", name="psproj")
                nc.tensor.matmul(
                    ps, lhsT=wout_sb[:, ot, :], rhs=attnT[:, tok0:tok0 + 512],
                    start=True, stop=True)
                if ot % 2 == 0:
                    nc.vector.tensor_copy(ob[:, ot, :], ps)
                else:
                    nc.scalar.activation(ob[:, ot, :], ps, FT.Copy)
                if ot % 2 == 1:
                    nc.gpsimd.dma_start(
                        out=outT_v[:, ot - 1:ot + 1, tok0:tok0 + 512],
                        in_=ob[:, ot - 1:ot + 1, :])

        # ---- schedule ----
        _qkv_start(0)
        _enqueue_chunk(0, 0, split=True)
        # wk/wv queued on scalar (q10) AFTER z chunk 0's odd halves so the
        # first q-projection's inputs arrive as early as possible
        nc.scalar.dma_start(out=wk_sb, in_=wk_v)
        nc.scalar.dma_start(out=wv_sb, in_=wv_v)
        drain_all()  # chunk 0 runs eagerly at startup
        prev = None
        for s in range(N * QC_PER_N):
            n, qc = s // QC_PER_N, s % QC_PER_N
            if s <= 2:
                # batch-0 ramp: alternate HW queues (sync->q1, scalar->q10)
                # to double the z feed rate
                _enqueue_chunk(0, s + 1,
                               queue=nc.scalar if s % 2 == 0 else nc.sync)
            if 3 <= s <= 14:
                b, c = (s - 3) // 4 + 1, (s - 3) % 4
                if c == 0:
                    _qkv_start(b)
                _enqueue_chunk(b, c,
                               queue=nc.scalar if c == 0 else nc.sync)
            if prev is not None:
                _enqueue_proj(*prev)
            # top up this step's filler from the proj backlog
            n_kt = 4 * qc + 4
            limit = (n_kt + 2) if s < 15 else 10 ** 9
            while proj_backlog and len(work) < limit:
                work.append(proj_backlog.pop(0))
            av, avfs = _attn_qc(n, qc)
            _attn_norm(n, qc, av, avfs)
            prev = (n, qc)
        work.extend(proj_backlog)
        proj_backlog.clear()
        drain_all()
        _proj_last(*prev)


_NC_CACHE = None


def _get_nc():
    global _NC_CACHE
    if _NC_CACHE is None:
        _NC_CACHE = build_bass()
    return _NC_CACHE


def _prepare_in_maps(z, Wqkv, Wout):
    zT = np.ascontiguousarray(z.reshape(NT, D).T).astype(ml_dtypes.bfloat16)
    scale = DK ** -0.5
    Wq = (Wqkv[:, :D] * scale).reshape(D, H, DK)
    Wk = Wqkv[:, D:2 * D].reshape(D, H, DK)
    Wv = Wqkv[:, 2 * D:].reshape(D, H, DK)
    tri = (np.arange(128)[None, :] >= np.arange(128)[:, None]).astype(
        ml_dtypes.bfloat16)
    in_maps = []
    for core in range(NC):
        h0 = HPC * core
        wq_c = np.ascontiguousarray(
            Wq[:, h0:h0 + HPC, :].reshape(D, HPC * DK)).astype(ml_dtypes.bfloat16)
        wk_c = np.ascontiguousarray(
            Wk[:, h0:h0 + HPC, :].reshape(D, HPC * DK)).astype(ml_dtypes.bfloat16)
        wv_c = np.ascontiguousarray(
            Wv[:, h0:h0 + HPC, :].reshape(D, HPC * DK)).astype(ml_dtypes.bfloat16)
        woutc = np.ascontiguousarray(
            Wout[core * 128:(core + 1) * 128, :]).astype(ml_dtypes.bfloat16)
        in_maps.append({
            "zT": zT, "wq": wq_c, "wk": wk_c, "wv": wv_c,
            "woutc": woutc, "tri": tri,
        })
    return in_maps


def _run(z, Wqkv, Wout, trace=False):
    nc = _get_nc()
    in_maps = _prepare_in_maps(z, Wqkv, Wout)
    res = run_bass_kernel_spmd(nc, in_maps, core_ids=list(range(NC)), trace=trace)
    acc = np.zeros((8, 128, NT), dtype=np.float32)
    for core in range(NC):
        acc += res.results[core]["outT"].astype(np.float32)
    out = acc.reshape(D, NT).T
    return np.ascontiguousarray(out).reshape(N, T, D), res


def kernel(z, Wqkv, Wout):
    out, _ = _run(np.asarray(z), np.asarray(Wqkv), np.asarray(Wout))
    return out
